# revision 24
# baseline (speedup 1.0000x reference)
"""Causal multi-head attention block (B=2, T=2048, D=1024, H=16) on 8 TRN2 cores.

Sharding: tensor-parallel over heads — each core owns 2 heads (128 cols of
w_attn's q/k/v blocks, 128 rows of w_proj) and produces a partial output
[B, T, D]; the host sums the 8 partials and adds the bias terms.

v3: single software-pipelined loop over the 8 (batch, q-chunk) tiles. All
matmul operands are bf16 (full PE rate at any free-dim width, half the
DMA/SBUF traffic; psum accumulation stays fp32). x is loaded to SBUF once.

The PE instruction queue is strictly in-order, and per k-tile the exp (ACT,
1.2 GHz) costs ~4/3 of the PE's score+AV work — so the PE stream is built so
it never waits on ACT:
  - AV matmuls lag the score matmuls by 2 k-tiles (probs buffered in SBUF);
  - the next chunk's QKV matmuls and the previous chunk's projection matmuls
    are chopped into ~0.5us pieces and spread between k-tiles as filler.
Per k-tile emission: [filler pieces] [av(kt-2)] [scores(kt)] [exp] [diag mask].

QKV:   qT,kT [128f, 512] = w^T @ x^T  (w stationary); v [512t, 128] = x @ w_v
       stored per (t-tile, head) as [128, 65] with a trailing ones column —
       the ones column makes the AV matmul also emit the softmax denominator.
attn:  per k-tile: sT [128k, w] = k @ qT for both heads concurrently (PE row
       groups 0-63 / 64-127), one flat exp per k-tile, {0,1} triangular mask
       mult only on the 128-wide diagonal subtile, AV accumulate [65, w].
norm:  row 64 of av = sum(exp); reciprocal -> broadcast -> scale rows 0..63.
proj:  out[128t, 512e] = a_stack^T @ w_proj rows; the last chunk uses
       per-head weights to start projecting the moment normalize lands.
"""
import numpy as np

import concourse.bass as bass
import concourse.mybir as mybir
import concourse.tile as tile
from concourse import bacc
from concourse.bass import ts, ds
from concourse.bass_utils import run_bass_kernel_spmd

F32 = mybir.dt.float32
BF16 = mybir.dt.bfloat16

B, T, D = 2, 2048, 1024
H = 16
HD = D // H          # 64
N_CORES = 8
HPC = H // N_CORES   # heads per core = 2
CW = HPC * HD        # per-core head width = 128
TCH = 512            # q/t chunk width
NTCH = (B * T) // TCH   # 8 chunks over flattened (b, t)
NKT = T // 128       # 16 k-tiles per batch
NQC = T // TCH       # 4 q-chunks per batch
DT = D // 128        # 8 d-tiles
LAG = 3              # AV matmuls trail score matmuls by this many k-tiles


def build_program(reps: int = 1, phases: str = "123"):
    nc = bacc.Bacc("TRN2", target_bir_lowering=False, debug=False,
                   num_devices=N_CORES)

    xT = nc.dram_tensor("xT", [B, D, T], BF16, kind="ExternalInput")
    wq = nc.dram_tensor("wq", [D, CW], BF16, kind="ExternalInput")
    wk = nc.dram_tensor("wk", [D, CW], BF16, kind="ExternalInput")
    wv = nc.dram_tensor("wv", [D, CW], BF16, kind="ExternalInput")
    bq = nc.dram_tensor("bq", [CW], F32, kind="ExternalInput")   # pre-scaled /8
    bk = nc.dram_tensor("bk", [CW], F32, kind="ExternalInput")
    wp = nc.dram_tensor("wp", [CW, D], BF16, kind="ExternalInput")
    mask = nc.dram_tensor("mask", [128, 128], BF16, kind="ExternalInput")
    ident = nc.dram_tensor("ident", [128, 128], BF16, kind="ExternalInput")
    out = nc.dram_tensor("out", [B, T, D], BF16, kind="ExternalOutput")

    with tile.TileContext(nc) as tc:
        with (
            tc.tile_pool(name="const", bufs=1) as const,
            tc.tile_pool(name="persist", bufs=1) as persist,
            tc.tile_pool(name="probs", bufs=5) as probs_pool,
            tc.tile_pool(name="norm", bufs=2) as norm_pool,
            tc.tile_pool(name="apool", bufs=2) as a_pool,
            tc.tile_pool(name="vt", bufs=2) as vt_pool,
            tc.tile_pool(name="osb", bufs=4) as osb_pool,
        ):
            # ---- constants ----
            wq_sb = const.tile([128, DT, CW], BF16)
            wk_sb = const.tile([128, DT, CW], BF16)
            wv_sb = const.tile([128, DT, CW], BF16)
            wp_sb = const.tile([128, D], BF16)
            wp2_sb = const.tile([HD, HPC, D], BF16)
            bq_sb = const.tile([128, 1], F32)
            bk_sb = const.tile([128, 1], F32)
            mask_sb = const.tile([128, 128], BF16)
            ident_sb = const.tile([128, 128], BF16)
            # consts go on the gpsimd DMA queue so the x chunks (sync queue)
            # land in parallel; first q/k d-tiles first for the fastest start
            wq_r = wq.rearrange("(dt p) m -> p dt m", p=128)
            wk_r = wk.rearrange("(dt p) m -> p dt m", p=128)
            nc.gpsimd.dma_start(wq_sb[:, 0:1, :], wq_r[:, 0:1, :])
            nc.gpsimd.dma_start(wk_sb[:, 0:1, :], wk_r[:, 0:1, :])
            nc.gpsimd.dma_start(wq_sb[:, 1:, :], wq_r[:, 1:, :])
            nc.gpsimd.dma_start(wk_sb[:, 1:, :], wk_r[:, 1:, :])
            nc.gpsimd.dma_start(wv_sb[:], wv.rearrange("(dt p) m -> p dt m", p=128))
            nc.gpsimd.dma_start(bq_sb[:], bq[:, None])
            nc.gpsimd.dma_start(bk_sb[:], bk[:, None])
            nc.gpsimd.dma_start(mask_sb[:], mask[:, :])
            nc.gpsimd.dma_start(ident_sb[:], ident[:, :])
            nc.gpsimd.dma_start(wp_sb[:], wp[:, :])
            nc.gpsimd.dma_start(wp2_sb[:], wp.rearrange("(h d) e -> d h e", h=HPC))

            # ---- persistent state ----
            # all of x stays in SBUF (64 KB/partition bf16) — loaded once
            x_sb = persist.tile([128, NTCH, DT, TCH], BF16)
            qT_sb = persist.tile([128, B * T], BF16)   # q/8, [2h*64, (b,t)]
            kT_sb = persist.tile([128, B * T], BF16)
            # v per t-tile & head: [v(64) | ones] — ones col makes the AV
            # matmul emit the softmax denominator in psum partition 64
            # per-(t-tile, head) unit padded to 128 elems so xbar-transpose
            # destinations are 256B-aligned; [v(64) | ones | pad(63)]
            v_sb = persist.tile([128, B * T // 128, HPC, 2 * HD], BF16)
            nc.vector.memset(v_sb[:, :, :, HD], 1.0)

            # preload the exp table set (~2.7us) so the first real exp
            # doesn't pay it mid-pipeline
            warm_i = const.tile([1, 8], F32)
            warm_o = const.tile([1, 8], F32)
            nc.vector.memset(warm_i[:], 0.0)
            nc.scalar.activation(warm_o[:], warm_i[:],
                                 mybir.ActivationFunctionType.Exp)

            # x load: chunk 0 split per d-tile so the first matmul starts
            # as soon as its slice lands; later chunks in halves
            for c in range(NTCH):
                b_i, qc = divmod(c, NQC)
                xsrc = xT[b_i].rearrange("(dt p) t -> p dt t", p=128)[
                    :, :, ds(qc * TCH, TCH)]
                gw = 1 if c == 0 else 4
                for g in range(DT // gw):
                    nc.sync.dma_start(x_sb[:, c, ts(g, gw), :],
                                      xsrc[:, ts(g, gw), :])

            def body(_=None):
                with (
                    tc.tile_pool(name="ps_s", bufs=2, space="PSUM") as ps_s,
                    tc.tile_pool(name="ps_av", bufs=2, space="PSUM") as ps_av,
                    tc.tile_pool(name="ps_m", bufs=2, space="PSUM") as ps_m,
                ):
                    def qkv_pieces(c, act_epi=False):
                        """QKV for chunk c as a list of ~0.5us PE pieces.
                        act_epi: run the q/k epilogues on ACT (Identity with
                        per-partition bias) when the hosting k-loop has ACT
                        slack — DVE is the tighter engine there."""
                        qcol = ds(c * TCH, TCH)
                        st = {}
                        P = []

                        def qk(g, which):
                            w_sb, b_sb = ((wq_sb, bq_sb) if which == "q"
                                          else (wk_sb, bk_sb))
                            if g == 0:
                                st[which] = ps_m.tile([128, TCH], F32, tag="m",
                                                      name=f"{which}_ps")
                            p = st[which]
                            for dt in range(2 * g, 2 * g + 2):
                                nc.tensor.matmul(p[:], w_sb[:, dt, :],
                                                 x_sb[:, c, dt, :],
                                                 start=dt == 0,
                                                 stop=dt == DT - 1)
                            if g == 3:
                                scale = 0.125 if which == "q" else 1.0
                                dst = qT_sb if which == "q" else kT_sb
                                if act_epi:
                                    nc.scalar.activation(
                                        dst[:, qcol], p[:],
                                        mybir.ActivationFunctionType.Identity,
                                        bias=b_sb[:], scale=scale)
                                elif which == "q":
                                    nc.vector.tensor_scalar(
                                        dst[:, qcol], p[:], 0.125, bq_sb[:],
                                        mybir.AluOpType.mult,
                                        mybir.AluOpType.add)
                                else:
                                    nc.vector.tensor_scalar_add(
                                        dst[:, qcol], p[:], bk_sb[:])

                        def vv(g):
                            # vT [128 vcol, 512 t]: weight-stationary matmuls
                            # like q/k — 8 big MMs instead of 32 small
                            # x-stationary ones (24 fewer serialized LDWs)
                            if g == 0:
                                st["v"] = ps_m.tile([128, TCH], F32, tag="m",
                                                    name="v_ps")
                                st["vt"] = vt_pool.tile([128, TCH], BF16,
                                                        name="vt_sb")
                            p = st["v"]
                            for dt in range(2 * g, 2 * g + 2):
                                nc.tensor.matmul(p[:], wv_sb[:, dt, :],
                                                 x_sb[:, c, dt, :],
                                                 start=dt == 0,
                                                 stop=dt == DT - 1)
                            if g == 3:
                                nc.vector.tensor_copy(st["vt"][:], p[:])

                        def vtr():
                            # full 128x128 PE transposes (the only working
                            # transpose shape) -> v[t, (h d)] layout, then one
                            # strided DVE copy into the padded v_sb units
                            vp = ps_m.tile([128, TCH // 128, HPC, HD], BF16,
                                           tag="m", name="vp_ps")
                            for s in range(TCH // 128):
                                nc.tensor.transpose(
                                    vp[:, s, :, :].rearrange("p h d -> p (h d)"),
                                    st["vt"][:, ts(s, 128)],
                                    ident_sb[:])
                            nc.vector.tensor_copy(
                                v_sb[:, ds(c * (TCH // 128), TCH // 128),
                                     :, 0:HD],
                                vp[:])

                        for g in range(4):
                            P.append(lambda g=g: qk(g, "q"))
                        for g in range(4):
                            P.append(lambda g=g: qk(g, "k"))
                        for g in range(4):
                            P.append(lambda g=g: vv(g))
                        P.append(vtr)
                        return P

                    def proj_pieces(a_t, c, qc_now):
                        """Projection of chunk c (uses a_t) as 4 PE pieces:
                        both e-halves of one t-tile per piece, so the two
                        matmuls share the stationary back-to-back."""
                        b_i, qc = divmod(c, NQC)
                        P = []

                        def pp(i, tt):
                            for ec in range(D // TCH):
                                o_ps = ps_m.tile([128, TCH], F32, tag="m",
                                                 name="o_ps")
                                nc.tensor.matmul(
                                    o_ps[:],
                                    a_t[:, ds((tt % 4) * 128, 128)],
                                    wp_sb[:, ts(ec, TCH)],
                                    start=True, stop=True)
                                o_sb = osb_pool.tile([128, TCH], BF16)
                                # ACT has slack in low-qc chunks; DVE is tight
                                if qc_now <= 1 and ec == 0:
                                    nc.scalar.copy(o_sb[:], o_ps[:])
                                else:
                                    nc.vector.tensor_copy(o_sb[:], o_ps[:])
                                nc.sync.dma_start(
                                    out[b_i, ts(tt, 128), ts(ec, TCH)],
                                    o_sb[:])

                        for i, tt in enumerate(range(qc * 4, qc * 4 + 4)):
                            P.append(lambda i=i, tt=tt: pp(i, tt))
                        return P

                    # chunk 0's QKV runs as a prologue block (ACT idle)
                    for p in qkv_pieces(0, act_epi=True):
                        p()

                    prev_a = None   # (a_tile, chunk) awaiting projection
                    for c in range(NTCH):
                        b_i, qc = divmod(c, NQC)
                        qcol = ds(c * TCH, TCH)
                        if "2" not in phases:
                            if c + 1 < NTCH:
                                for p in qkv_pieces(c + 1):
                                    p()
                            continue

                        filler = []
                        if c + 1 < NTCH:
                            filler += qkv_pieces(c + 1, act_epi=qc <= 1)
                        if prev_a is not None and "3" in phases:
                            filler += proj_pieces(prev_a[0], prev_a[1], qc)
                            prev_a = None

                        nkt = 4 * qc + 4
                        avs = [ps_av.tile([HD + 1, TCH], F32, tag="av",
                                          name=f"av{_h}")
                               for _h in range(HPC)]
                        pps = [None] * nkt

                        def av_mms(kt):
                            j = kt - 4 * qc
                            f0 = max(j, 0) * 128
                            fsl = ds(f0, TCH - f0)
                            for h in range(HPC):
                                nc.tensor.matmul(
                                    avs[h][:, fsl],
                                    v_sb[:, b_i * NKT + kt, h, 0:HD + 1],
                                    pps[kt][:, h, fsl],
                                    start=(kt == 0), stop=(kt == nkt - 1))

                        done = 0
                        for kt in range(nkt):
                            want = (len(filler) * (kt + 1)) // nkt
                            while done < want:
                                filler[done]()
                                done += 1
                            if kt >= LAG:
                                av_mms(kt - LAG)
                            j = kt - 4 * qc   # >=0: diagonal-straddling tile
                            f0 = max(j, 0) * 128
                            fsl = ds(f0, TCH - f0)
                            ktcol = ds(b_i * T + kt * 128, 128)
                            sp_ps = ps_s.tile([128, HPC, TCH], F32)
                            for h in range(HPC):
                                hp = ds(h * HD, HD)
                                nc.tensor.matmul(
                                    sp_ps[:, h, fsl],
                                    kT_sb[hp, ktcol],
                                    qT_sb[hp, qcol][:, fsl],
                                    start=True, stop=True)
                            pp_sb = probs_pool.tile([128, HPC, TCH], BF16)
                            pps[kt] = pp_sb
                            if f0 == 0:
                                nc.scalar.activation(
                                    pp_sb.rearrange("p h w -> p (h w)"),
                                    sp_ps.rearrange("p h w -> p (h w)"),
                                    mybir.ActivationFunctionType.Exp)
                            else:
                                nc.scalar.activation(
                                    pp_sb[:, :, fsl], sp_ps[:, :, fsl],
                                    mybir.ActivationFunctionType.Exp)
                            if j >= 0:
                                dsl = ds(j * 128, 128)
                                for h in range(HPC):
                                    nc.vector.tensor_tensor(
                                        pp_sb[:, h, dsl], pp_sb[:, h, dsl],
                                        mask_sb[:, :], mybir.AluOpType.mult)
                        for kt in range(max(nkt - LAG, 0), nkt):
                            av_mms(kt)

                        # ============ normalize chunk c ============
                        last = c == NTCH - 1
                        a_t = a_pool.tile([128, TCH], BF16, name="a_t")
                        at_tiles = []
                        for h in range(HPC):
                            av_ps = avs[h]
                            r64_sb = norm_pool.tile([HD + 1, TCH], F32,
                                                    tag="r64")
                            nc.vector.reciprocal(
                                r64_sb[HD:HD + 1, :], av_ps[HD:HD + 1, :])
                            # partition_broadcast needs its source at
                            # physical partition 0 — DMA-shift it down
                            r0_sb = norm_pool.tile([1, TCH], F32, tag="r0")
                            nc.sync.dma_start(r0_sb[:], r64_sb[HD:HD + 1, :])
                            bc_sb = norm_pool.tile([HD, TCH], F32, tag="bc")
                            nc.gpsimd.partition_broadcast(bc_sb[:], r0_sb[:])
                            if last:
                                # per-head tile, read directly by the tail
                                # projection (no partition shift on the
                                # critical path)
                                at_sb = norm_pool.tile([HD, TCH], BF16,
                                                       tag=f"at{h}")
                                nc.vector.tensor_tensor(
                                    at_sb[:], av_ps[0:HD, :], bc_sb[:],
                                    mybir.AluOpType.mult)
                                at_tiles.append(at_sb)
                            elif h == 0:
                                # lanes aligned: write straight into a_t
                                nc.vector.tensor_tensor(
                                    a_t[0:HD, :], av_ps[0:HD, :], bc_sb[:],
                                    mybir.AluOpType.mult)
                            else:
                                at_sb = norm_pool.tile([HD, TCH], BF16,
                                                       tag="at_hi")
                                nc.vector.tensor_tensor(
                                    at_sb[:], av_ps[0:HD, :], bc_sb[:],
                                    mybir.AluOpType.mult)
                                nc.sync.dma_start(a_t[ds(HD, HD), :], at_sb[:])

                        if "3" not in phases:
                            continue
                        if not last:
                            prev_a = (a_t, c)
                        else:
                            # tail: per-head weights so each (tt, ec) starts
                            # as soon as the normalize lands; psum via the
                            # 2-slot misc rotation + av slots
                            for i, (tt, ec) in enumerate(
                                    (t_, e_)
                                    for t_ in range(qc * 4, qc * 4 + 4)
                                    for e_ in range(D // TCH)):
                                if i % 2 == 1:
                                    o_ps = ps_av.tile([128, TCH], F32,
                                                      tag="av", name="o_av")
                                else:
                                    o_ps = ps_m.tile([128, TCH], F32, tag="m",
                                                     name="o_ps")
                                tloc = ds((tt - qc * 4) * 128, 128)
                                for h in range(HPC):
                                    nc.tensor.matmul(
                                        o_ps[:],
                                        at_tiles[h][:, tloc],
                                        wp2_sb[:, h, ts(ec, TCH)],
                                        start=(h == 0), stop=(h == HPC - 1))
                                o_sb = osb_pool.tile([128, TCH], BF16)
                                nc.vector.tensor_copy(o_sb[:], o_ps[:])
                                nc.sync.dma_start(
                                    out[b_i, ts(tt, 128), ts(ec, TCH)],
                                    o_sb[:])

            if reps == 1:
                body()
            else:
                with tc.For_i(0, reps, 1) as _i:
                    body(_i)

    nc.compile()
    return nc


def make_mask() -> np.ndarray:
    """Triangular multiplicative mask for the 128-wide diagonal subtile:
    keep (p <= f)."""
    import ml_dtypes
    p = np.arange(128)[:, None]
    f = np.arange(128)[None, :]
    return (p <= f).astype(ml_dtypes.bfloat16)


def make_in_maps(x, w_attn, b_attn, w_proj):
    import ml_dtypes
    bf16 = ml_dtypes.bfloat16
    xT = np.ascontiguousarray(
        np.transpose(x, (0, 2, 1)), dtype=np.float32).astype(bf16)
    mask = make_mask()
    in_maps = []
    for c in range(N_CORES):
        cs = slice(CW * c, CW * (c + 1))
        in_maps.append({
            "xT": xT,
            "wq": np.ascontiguousarray(w_attn[:, 0 * D:1 * D][:, cs]).astype(bf16),
            "wk": np.ascontiguousarray(w_attn[:, 1 * D:2 * D][:, cs]).astype(bf16),
            "wv": np.ascontiguousarray(w_attn[:, 2 * D:3 * D][:, cs]).astype(bf16),
            "bq": np.ascontiguousarray(b_attn[0 * D:1 * D][cs]) * 0.125,
            "bk": np.ascontiguousarray(b_attn[1 * D:2 * D][cs]),
            "wp": np.ascontiguousarray(w_proj[cs, :]).astype(bf16),
            "mask": mask,
            "ident": np.eye(128).astype(bf16),
        })
    return in_maps


def host_bias(b_attn, b_proj, w_proj):
    # v-bias propagates exactly through softmax (rows sum to 1) and the linear
    # projection: out += b_v @ w_proj + b_proj
    return b_proj.astype(np.float32) + b_attn[2 * D:3 * D].astype(np.float32) @ w_proj.astype(np.float32)


_NC_CACHE = {}


def get_program(reps: int = 1, phases: str = "123"):
    key = (reps, phases)
    if key not in _NC_CACHE:
        _NC_CACHE[key] = build_program(reps, phases)
    return _NC_CACHE[key]


def kernel(x, w_attn, b_attn, w_proj, b_proj):
    x = np.asarray(x, np.float32)
    w_attn = np.asarray(w_attn, np.float32)
    b_attn = np.asarray(b_attn, np.float32)
    w_proj = np.asarray(w_proj, np.float32)
    b_proj = np.asarray(b_proj, np.float32)

    nc = get_program()
    in_maps = make_in_maps(x, w_attn, b_attn, w_proj)
    res = run_bass_kernel_spmd(nc, in_maps, core_ids=list(range(N_CORES)))
    acc = np.zeros((B, T, D), np.float64)
    for r in res.results:
        acc += r["out"].astype(np.float64)
    acc += host_bias(b_attn, b_proj, w_proj).astype(np.float64)
    return acc.astype(np.float32)
